# revision 1
# baseline (speedup 1.0000x reference)
"""Trainium2 Bass kernel for nn_CustomLoss_38096359916115.

Computes, over y, y_echo, f of shape (8192, 4096) and scalar mu in (0,1):
  pseudo_l0_loss = sum((|y| >= mu) + (0 < |y| < mu) * |y| / mu) / n
                 = sum(min(|y|, mu)) / (mu * n)          (exact identity)
  l2_loss        = sum((y_echo / 22.8 - f)^2) / n

Strategy (data-parallel, memory-bound):
  - Shard the n=8192 row dimension across 8 NeuronCores (1024 rows each).
  - Per core, stream 8 blocks of [128, 4096] f32 per tensor from HBM
    (2 MiB contiguous HWDGE DMAs, double-buffered).
    L0 path: DVE tensor_scalar clamp(y, -mu, mu) in place, then ScalarE
             Abs activation with fused accum_out  -> sum(min(|y|, mu)).
    L2 path: DVE scalar_tensor_tensor (y_echo * (1/22.8)) - f in place,
             then ScalarE Square activation with fused accum_out.
  - Each block's two partial sums land in columns of a [128, 16] SBUF
    accumulator; one tiny DMA returns it per core.
  - Host sums the 8 x [128, 16] partials in float64 and applies the
    1/(mu*n) and 1/n scalings.

The program is traced under TileContext on a bacc.Bacc and then
nc.compile()d: the generate_event_semaphores pass splits multi-wait
instructions (TRN2 allows one sync wait per instruction) and
codegen_inst_isa_subclasses produces valid ISA encodings.
"""

import numpy as np

_ECHO_SCALE = 22.8
_P = 128
_N, _M = 8192, 4096
_NCORES = 8
_ROWS = _N // _NCORES  # rows per core
_NT = _ROWS // _P      # [128, _M] blocks per core

_cache = {}


def _ensure_path():
    try:
        import concourse  # noqa: F401
    except ImportError:
        import sys

        for p in ("/opt/trn_rl_repo", "/opt/pypackages"):
            if p not in sys.path:
                sys.path.append(p)
        import concourse  # noqa: F401


_CHUNK = 1024  # accum fold length; shorter folds -> smaller fp32 sum error


def build(rows=_ROWS, cols=_M, n_cores=_NCORES, bufs=3):
    """Trace + compile the per-core program. Returns (nc, n_tiles)."""
    _ensure_path()
    import concourse.mybir as mybir
    import concourse.tile as tile
    from concourse import bacc

    f32 = mybir.dt.float32
    Alu = mybir.AluOpType
    Act = mybir.ActivationFunctionType
    nt = rows // _P
    nch = max(1, cols // _CHUNK)  # accum chunks per tile
    chunk = cols // nch
    half = nt * nch  # accumulator columns per loss

    nc = bacc.Bacc(
        "TRN2", target_bir_lowering=False, debug=False, num_devices=n_cores
    )
    y = nc.dram_tensor("y", [rows, cols], f32, kind="ExternalInput").ap()
    ye = nc.dram_tensor("y_echo", [rows, cols], f32, kind="ExternalInput").ap()
    ff = nc.dram_tensor("f", [rows, cols], f32, kind="ExternalInput").ap()
    # column 0: mu, column 1: -mu
    mu2_b = nc.dram_tensor("mu2_b", [_P, 2], f32, kind="ExternalInput").ap()
    out = nc.dram_tensor("partials", [_P, 2 * half], f32, kind="ExternalOutput").ap()

    yt = y.rearrange("(n p) m -> n p m", p=_P)
    yet = ye.rearrange("(n p) m -> n p m", p=_P)
    fft = ff.rearrange("(n p) m -> n p m", p=_P)

    with tile.TileContext(nc) as tc:
        with (
            tc.tile_pool(name="consts", bufs=1) as cpool,
            tc.tile_pool(name="ldy", bufs=bufs) as ypool,
            tc.tile_pool(name="lde", bufs=bufs) as epool,
            tc.tile_pool(name="ldf", bufs=bufs) as fpool,
        ):
            mu_t = cpool.tile([_P, 2], f32)
            nc.sync.dma_start(mu_t[:], mu2_b[:])
            acc = cpool.tile([_P, 2 * half], f32)

            for i in range(nt):
                ty = ypool.tile([_P, cols], f32)
                nc.sync.dma_start(ty[:], yt[i])
                te = epool.tile([_P, cols], f32)
                nc.sync.dma_start(te[:], yet[i])
                tf = fpool.tile([_P, cols], f32)
                nc.sync.dma_start(tf[:], fft[i])

                # ty = clamp(y, -mu, mu); |clamp| == min(|y|, mu)
                nc.vector.tensor_scalar(
                    ty[:], ty[:], mu_t[:, 0:1], mu_t[:, 1:2], Alu.min, Alu.max
                )
                # te = (y_echo * (1/22.8)) - f
                nc.vector.scalar_tensor_tensor(
                    te[:], te[:], 1.0 / _ECHO_SCALE, tf[:], Alu.mult, Alu.subtract
                )
                # accum per chunk: sum(|clamp|) and sum(diff^2) along free dim
                for j in range(nch):
                    cs = slice(j * chunk, (j + 1) * chunk)
                    col = i * nch + j
                    nc.scalar.activation(
                        ty[:, cs],
                        ty[:, cs],
                        Act.Abs,
                        accum_out=acc[:, col : col + 1],
                    )
                    nc.scalar.activation(
                        te[:, cs],
                        te[:, cs],
                        Act.Square,
                        accum_out=acc[:, half + col : half + col + 1],
                    )

            nc.sync.dma_start(out[:], acc[:])

    nc.compile()
    return nc, nt


def _get_nc():
    if "nc" not in _cache:
        _cache["nc"] = build()
    return _cache["nc"]


def make_in_maps(y, y_echo, f, mu, rows=_ROWS, n_cores=_NCORES):
    mu_f = float(np.asarray(mu).reshape(-1)[0])
    mu2 = np.empty((_P, 2), np.float32)
    mu2[:, 0] = mu_f
    mu2[:, 1] = -mu_f
    in_maps = []
    for c in range(n_cores):
        sl = slice(c * rows, (c + 1) * rows)
        in_maps.append(
            {
                "y": np.ascontiguousarray(y[sl]),
                "y_echo": np.ascontiguousarray(y_echo[sl]),
                "f": np.ascontiguousarray(f[sl]),
                "mu2_b": mu2,
            }
        )
    return in_maps


def reduce_partials(partials, mu):
    """partials: list/array of per-core [128, 2*half] f32 -> (l0, l2) f32."""
    mu_f = float(np.asarray(mu).reshape(-1)[0])
    parts = np.asarray(partials, dtype=np.float64)
    half = parts.shape[-1] // 2
    s0 = parts[..., :half].sum()
    s1 = parts[..., half:].sum()
    l0 = np.float32(s0 / (mu_f * _N))
    l2 = np.float32(s1 / _N)
    return np.asarray(l0, np.float32), np.asarray(l2, np.float32)


def kernel(y, y_echo, f, mu):
    _ensure_path()
    from concourse.bass_utils import run_bass_kernel_spmd

    nc, nt = _get_nc()
    in_maps = make_in_maps(y, y_echo, f, mu)
    res = run_bass_kernel_spmd(nc, in_maps, list(range(_NCORES))).results
    partials = [r["partials"] for r in res]
    return reduce_partials(partials, mu)



# revision 29
# speedup vs baseline: 1.0620x; 1.0620x over previous
"""Trainium2 Bass kernel for nn_CustomLoss_38096359916115.

Computes, over y, y_echo, f of shape (8192, 4096) and scalar mu in (0,1):
  pseudo_l0_loss = sum((|y| >= mu) + (0 < |y| < mu) * |y| / mu) / n
                 = sum(min(|y|, mu)) / (mu * n)          (exact identity)
  l2_loss        = sum((y_echo / 22.8 - f)^2) / n

Strategy (data-parallel, memory-bound; per-core DMA roofline ~140 us):
  - Shard the n=8192 row dimension across 8 NeuronCores (1024 rows each).
  - mu is baked into the program as an immediate (compile cached per mu
    value), so no mu DMA and no SBUF broadcast tile.
  - Per core, stream [128, 4096] f32 row-blocks of each tensor from HBM
    (2 MiB contiguous HWDGE DMAs on the sync queue, triple-buffered).
    L2 path: DVE scalar_tensor_tensor  (y_echo * (1/22.8)) - f, then one
             ScalarE Square activation with fused accum_out.
    L0 path: DVE clamp(y, -mu, mu), then ScalarE Abs activation with
             fused accum_out -> sum(min(|y|, mu)).
  - The LAST row-block is processed as column strips so its compute
    pipelines with the trailing DMAs instead of serializing after the
    final transfer: three 1024-col strips from small pools, then two
    512-col strips in dedicated buffers (no write-after-read waits).
    SP SEQ needs ~800 ns per DMA issue (decode + HWDGE held on SEQ), so
    strips narrower than ~600 cols would be issue-bound - 1024 keeps
    the DMA engines fed.
  - For the two FINAL strips the L0 abs-sum uses the exact identity
    sum|t| = sum max(t,0) - sum min(t,0) as two DVE tensor_scalar
    add-reductions (P and N columns), keeping the post-last-transfer
    critical chain on the fast DVE path instead of the ACT queue.
    (tensor_scalar's accum_out form treats op1 as the REDUCTION op;
    abs_max is not a valid ALU op for it on real HW.)
  - Partial sums land in disjoint columns of one [128, C] SBUF
    accumulator; a single small DMA returns it per core. The host sums
    the 8 x [128, C] partials in float64 and applies the final scales.
"""

import numpy as np

_ECHO_SCALE = 22.8
_P = 128
_N, _M = 8192, 4096
_NCORES = 8
_ROWS = _N // _NCORES  # rows per core
_NSTRIPS = 4           # wide strips for the last row-block (last one split)

_cache = {}


def _ensure_path():
    try:
        import concourse  # noqa: F401
    except ImportError:
        import sys

        for p in ("/opt/trn_rl_repo", "/opt/pypackages"):
            if p not in sys.path:
                sys.path.append(p)
        import concourse  # noqa: F401


def _layout(rows=_ROWS, cols=_M, n_strips=_NSTRIPS):
    """Accumulator column layout shared by build() and reduce_partials().

    Returns (widths, abs_cols, p_cols, n_cols, sq_cols, total_cols):
    widths = strip widths of the last row-block; abs_cols hold ACT
    Abs-accum L0 sums, p_cols/n_cols the P/N L0 split of the final
    strips, sq_cols the L2 sums.
    """
    nt = rows // _P
    sw = cols // n_strips
    # graded final strips: each pre-final strip's compute chain clears
    # before the last strip's data lands
    widths = [sw] * (n_strips - 1) + [sw // 2, sw // 4, sw - sw // 2 - sw // 4]
    n_fin = 3
    abs_cols, p_cols, n_cols, sq_cols = [], [], [], []
    c = 0
    # full row-blocks and wide strips use ACT Abs-accum (one L0 col);
    # the three dedicated final strips use the DVE P/N split (two L0
    # cols) to keep the post-transfer chain off the busier ACT queue
    for _ in range((nt - 1) + (n_strips - 1)):
        abs_cols.append(c)
        sq_cols.append(c + 1)
        c += 2
    for _ in range(n_fin):
        p_cols.append(c)
        n_cols.append(c + 1)
        sq_cols.append(c + 2)
        c += 3
    return widths, abs_cols, p_cols, n_cols, sq_cols, c


def build(mu, rows=_ROWS, cols=_M, n_cores=_NCORES, bufs=3, n_strips=_NSTRIPS):
    """Trace + compile the per-core program for a fixed mu immediate."""
    _ensure_path()
    import concourse.mybir as mybir
    import concourse.tile as tile
    from concourse import bacc

    f32 = mybir.dt.float32
    Alu = mybir.AluOpType
    Act = mybir.ActivationFunctionType
    mu = float(mu)
    inv_s = 1.0 / _ECHO_SCALE
    nt = rows // _P
    widths, abs_cols, p_cols, n_cols, sq_cols, ncols_acc = _layout(
        rows, cols, n_strips
    )

    nc = bacc.Bacc(
        "TRN2", target_bir_lowering=False, debug=False, num_devices=n_cores
    )
    y = nc.dram_tensor("y", [rows, cols], f32, kind="ExternalInput").ap()
    ye = nc.dram_tensor("y_echo", [rows, cols], f32, kind="ExternalInput").ap()
    ff = nc.dram_tensor("f", [rows, cols], f32, kind="ExternalInput").ap()
    out = nc.dram_tensor("partials", [_P, ncols_acc], f32, kind="ExternalOutput").ap()

    yt = y.rearrange("(n p) m -> n p m", p=_P)
    yet = ye.rearrange("(n p) m -> n p m", p=_P)
    fft = ff.rearrange("(n p) m -> n p m", p=_P)

    def col(c):
        return acc[:, c : c + 1]

    with tile.TileContext(nc) as tc:
        with (
            tc.tile_pool(name="consts", bufs=1) as cpool,
            tc.tile_pool(name="ldy", bufs=bufs) as ypool,
            tc.tile_pool(name="lde", bufs=bufs) as epool,
            tc.tile_pool(name="ldf", bufs=bufs) as fpool,
            tc.tile_pool(name="sldy", bufs=bufs) as sypool,
            tc.tile_pool(name="slde", bufs=bufs) as sepool,
            tc.tile_pool(name="sldf", bufs=bufs) as sfpool,
        ):
            acc = cpool.tile([_P, ncols_acc], f32)
            blk = 0  # running block index into abs_cols/sq_cols

            for i in range(nt - 1):
                ty = ypool.tile([_P, cols], f32)
                nc.sync.dma_start(ty[:], yt[i])
                te = epool.tile([_P, cols], f32)
                nc.sync.dma_start(te[:], yet[i])
                tf = fpool.tile([_P, cols], f32)
                nc.sync.dma_start(tf[:], fft[i])

                # te = (y_echo * (1/22.8)) - f, then ACT squares+accums
                nc.vector.scalar_tensor_tensor(
                    te[:], te[:], inv_s, tf[:], Alu.mult, Alu.subtract
                )
                nc.scalar.activation(
                    te[:], te[:], Act.Square, accum_out=col(sq_cols[blk])
                )
                # ty = clamp(y, -mu, mu), then ACT abs+accums
                nc.vector.tensor_scalar(ty[:], ty[:], mu, -mu, Alu.min, Alu.max)
                nc.scalar.activation(
                    ty[:], ty[:], Act.Abs, accum_out=col(abs_cols[blk])
                )
                blk += 1

            # Last row-block: column strips, loads ordered (e, f, y), all
            # issued from SP.
            c0 = 0
            for j, w in enumerate(widths):
                cs = slice(c0, c0 + w)
                c0 += w
                fin = j >= len(widths) - 3  # final dedicated strips
                if not fin:
                    se = sepool.tile([_P, w], f32)
                    sf = sfpool.tile([_P, w], f32)
                    sy = sypool.tile([_P, w], f32)
                else:
                    se = cpool.tile([_P, w], f32, tag=f"fin_e{j}")
                    sf = cpool.tile([_P, w], f32, tag=f"fin_f{j}")
                    sy = cpool.tile([_P, w], f32, tag=f"fin_y{j}")
                nc.sync.dma_start(se[:], yet[nt - 1][:, cs])
                nc.sync.dma_start(sf[:], fft[nt - 1][:, cs])
                nc.sync.dma_start(sy[:], yt[nt - 1][:, cs])

                nc.vector.scalar_tensor_tensor(
                    se[:], se[:], inv_s, sf[:], Alu.mult, Alu.subtract
                )
                nc.vector.tensor_scalar(sy[:], sy[:], mu, -mu, Alu.min, Alu.max)
                nc.scalar.activation(
                    se[:], se[:], Act.Square, accum_out=col(sq_cols[blk])
                )
                if not fin:
                    nc.scalar.activation(
                        sy[:], sy[:], Act.Abs, accum_out=col(abs_cols[blk])
                    )
                else:
                    # sum|clamp| = sum max(clamp,0) - sum min(clamp,0) on
                    # DVE (short tail chain; op1 is the reduce op here).
                    # P dumps into sf (its last reader, stt, ran earlier).
                    m = j - (len(widths) - 3)
                    nc.vector.tensor_scalar(
                        sf[:], sy[:], 0.0, None, Alu.max, Alu.add,
                        accum_out=col(p_cols[m]),
                    )
                    nc.vector.tensor_scalar(
                        sy[:], sy[:], 0.0, None, Alu.min, Alu.add,
                        accum_out=col(n_cols[m]),
                    )
                blk += 1

            # Split output: the bulk leaves while the final strip still
            # computes; only the last strip's 3 columns ride the tail.
            nc.sync.dma_start(out[:, : ncols_acc - 3], acc[:, : ncols_acc - 3])
            nc.sync.dma_start(out[:, ncols_acc - 3 :], acc[:, ncols_acc - 3 :])

    nc.compile()
    return nc, ncols_acc


def _get_nc(mu_f):
    key = float(np.float32(mu_f))
    if key not in _cache:
        _cache[key] = build(key)
    return _cache[key]


def make_in_maps(y, y_echo, f, rows=_ROWS, n_cores=_NCORES):
    in_maps = []
    for c in range(n_cores):
        sl = slice(c * rows, (c + 1) * rows)
        in_maps.append(
            {
                "y": np.ascontiguousarray(y[sl]),
                "y_echo": np.ascontiguousarray(y_echo[sl]),
                "f": np.ascontiguousarray(f[sl]),
            }
        )
    return in_maps


def reduce_partials(partials, mu, rows=_ROWS, cols=_M, n_strips=_NSTRIPS):
    """partials: per-core [128, C] f32 arrays -> (l0, l2) f32."""
    mu_f = float(np.asarray(mu).reshape(-1)[0])
    _, abs_cols, p_cols, n_cols, sq_cols, _ = _layout(rows, cols, n_strips)
    parts = np.asarray(partials, dtype=np.float64)
    s0 = (
        parts[..., abs_cols].sum()
        + parts[..., p_cols].sum()
        - parts[..., n_cols].sum()
    )
    s1 = parts[..., sq_cols].sum()
    n = rows * len(partials)
    l0 = np.float32(s0 / (mu_f * n))
    l2 = np.float32(s1 / n)
    return np.asarray(l0, np.float32), np.asarray(l2, np.float32)


def kernel(y, y_echo, f, mu):
    _ensure_path()
    from concourse.bass_utils import run_bass_kernel_spmd

    mu_f = float(np.asarray(mu).reshape(-1)[0])
    nc, _ = _get_nc(mu_f)
    in_maps = make_in_maps(y, y_echo, f)
    res = run_bass_kernel_spmd(nc, in_maps, list(range(_NCORES))).results
    partials = [r["partials"] for r in res]
    return reduce_partials(partials, mu)
